# revision 15
# baseline (speedup 1.0000x reference)
"""DifferentiableRAM (DRAW-style attention read) Trainium2 Bass kernel.

Reference computation (per batch b, channel c):
    gx = W*(p0+1)/2, gy = H*(p1+1)/2, sigma2 = exp(p2),
    delta = exp(p3)*(W-1)/(N-1), gamma = exp(p4)
    mu[i]  = g + delta*(i - N/2 - 0.5)                      i in [0,N)
    F[i,a] = exp(-(a-mu[i])^2 / (2 sigma2)) ;  Fn = F / (F.sum(a) + 1e-4)
    out[b,c] = gamma * Fy_n @ x[b,c] @ Fx_n^T                [N, N]

Strategy: pure data parallel over batch (B=32 -> 4 per core on 8 cores).
Both filterbanks are generated transposed, T[a, i] = exp(nhs*(a-mu_i)^2),
y and x halves side by side in one [128, 512] tile per 128-row chunk, so
both GEMMs contract along the partition axis:
    G1: FyxT[w, n] = sum_h x[h, w] * Ty[h, n]      (lhsT = x chunk)
    G2: raw[n, m]  = sum_w FyxT[w, n] * Tx[w, m]   (lhsT = FyxT chunk)
    out[n, m] = raw[n, m] * (gamma * invy[n]) * invx[m]
GEMMs run in fp16 (full PE rate).  Normalizers stay fp32 and are applied
to the fp32 PSUM of G2.

Performance structure (driven by the TimelineSim cost model):
  - one big DMA per (b,c) x tile and per (b,c) output tile (25 DMAs
    total) to keep the SP sequencer + HWDGE dispatch path off the
    critical path; all 12 x loads are issued up front so DMA streams
    back-to-back,
  - output stored as fp16 (host casts back to fp32) to halve store
    traffic,
  - mu rows broadcast to all partitions with gpsimd partition_broadcast
    (idle Pool engine) instead of fp32 PE matmuls,
  - d = mu - a is folded into the Square activation via a per-partition
    bias column, T = Exp(nhs * Square(mu + bias)),
  - PSUM->SBUF copies on Pool, fp32->fp16 input conversion split across
    DVE/Act/Pool, output scaling on DVE,
  - a few zero-weight matmuls prime the PE p-state ramp while the first
    tiles load.
"""

import numpy as np
from contextlib import ExitStack

import concourse.tile as tile
from concourse import bacc, mybir
from concourse.bass_utils import run_bass_kernel_spmd

F32 = mybir.dt.float32
F16 = mybir.dt.float16
ALU = mybir.AluOpType
ACTF = mybir.ActivationFunctionType

B, C, H, W = 32, 3, 512, 512
N = 256
NCORES = 8
BL = B // NCORES  # batches per core
KC = 4            # 128-row chunks of the 512-long axis
SMALL = 1e-4
DELTA_SCALE = (max(W, H) - 1) / (N - 1.0)
NPRIME = 16       # PE p-state priming matmuls


def _kernel_body(tc):
    nc = tc.nc
    x_d = nc.dram_tensor("x", [BL, C, H, W], F32, kind="ExternalInput").ap()
    p_d = nc.dram_tensor("p", [BL, 5], F32, kind="ExternalInput").ap()
    o_d = nc.dram_tensor("out", [BL, C, N, N], F16, kind="ExternalOutput").ap()

    with ExitStack() as ctx:
        consts = ctx.enter_context(tc.tile_pool(name="consts", bufs=1))
        params = ctx.enter_context(tc.tile_pool(name="params", bufs=1))
        xf32p = ctx.enter_context(tc.tile_pool(name="xf32p", bufs=12))
        xbfp = ctx.enter_context(tc.tile_pool(name="xbfp", bufs=6))
        tban = ctx.enter_context(tc.tile_pool(name="tban", bufs=8))
        bcp = ctx.enter_context(tc.tile_pool(name="bcp", bufs=4))
        sqtmp = ctx.enter_context(tc.tile_pool(name="sqtmp", bufs=3))
        fyxp = ctx.enter_context(tc.tile_pool(name="fyxp", bufs=4))
        outp = ctx.enter_context(tc.tile_pool(name="outp", bufs=3))
        rows = ctx.enter_context(tc.tile_pool(name="rows", bufs=4))
        colp = ctx.enter_context(tc.tile_pool(name="colp", bufs=4))
        invp = ctx.enter_context(tc.tile_pool(name="invp", bufs=2))
        # PSUM: 8 banks - ps1 4 + ps2 2 + pscs 1 + pscol 1
        ps1 = ctx.enter_context(tc.tile_pool(name="ps1", bufs=4, space="PSUM"))
        ps2 = ctx.enter_context(tc.tile_pool(name="ps2", bufs=2, space="PSUM"))
        pscs = ctx.enter_context(tc.tile_pool(name="pscs", bufs=1, space="PSUM"))
        pscol = ctx.enter_context(tc.tile_pool(name="pscol", bufs=1, space="PSUM"))

        # ---- constants -------------------------------------------------
        a_iota = consts.tile([128, 1], F32)  # partition index 0..127
        nc.gpsimd.iota(a_iota, pattern=[[0, 1]], base=0, channel_multiplier=1,
                       allow_small_or_imprecise_dtypes=True)
        # nbias[p, k] = -(p + 128k): per-partition bias for the Square
        nbias = consts.tile([128, KC], F32)
        nc.gpsimd.iota(nbias, pattern=[[-128, KC]], base=0,
                       channel_multiplier=-1,
                       allow_small_or_imprecise_dtypes=True)
        iota4 = consts.tile([BL, 2 * N], F32)  # 0..255 twice, on BL partitions
        nc.gpsimd.iota(iota4, pattern=[[0, 2], [1, N]], base=0,
                       channel_multiplier=0, allow_small_or_imprecise_dtypes=True)
        ones_k = consts.tile([128, 1], F16)  # colsum lhsT
        nc.vector.memset(ones_k, 1.0)
        one1 = consts.tile([1, 1], F32)       # row->col rhs
        nc.vector.memset(one1, 1.0)
        small1 = consts.tile([1, 1], F32)     # filterbank-normalizer epsilon
        nc.vector.memset(small1, SMALL)
        czero = consts.tile([128, 2 * N], F16)  # PE p-state primer operand
        nc.vector.memset(czero, 0.0)

        # ---- per-batch attention params (partition = batch) ------------
        pt = params.tile([BL, 5], F32)
        nc.sync.dma_start(out=pt, in_=p_d)
        E = params.tile([BL, 3], F32)  # [sigma2, exp(p3), gamma]
        nc.scalar.activation(E, pt[:, 2:5], ACTF.Exp)
        en = params.tile([BL, 1], F32)  # exp(-p2) = 1/sigma2
        nc.scalar.activation(en, pt[:, 2:3], ACTF.Exp, scale=-1.0)
        delta = params.tile([BL, 1], F32)
        nc.vector.tensor_scalar(delta, E[:, 1:2], DELTA_SCALE, None, ALU.mult)
        g2 = params.tile([BL, 2], F32)  # [gx, gy]
        nc.vector.tensor_scalar(g2, pt[:, 0:2], W / 2.0, W / 2.0, ALU.mult, ALU.add)
        cyx = params.tile([BL, 2], F32)  # g - (N/2+0.5)*delta ; [:,0]=y uses gy
        nc.vector.scalar_tensor_tensor(cyx[:, 0:1], delta, -(N / 2.0 + 0.5),
                                       g2[:, 1:2], ALU.mult, ALU.add)
        nc.vector.scalar_tensor_tensor(cyx[:, 1:2], delta, -(N / 2.0 + 0.5),
                                       g2[:, 0:1], ALU.mult, ALU.add)
        # per-batch row: [mu_y (N) | mu_x (N) | nhs | gamma]
        M4 = params.tile([BL, 2 * N + 2], F32)
        nc.vector.tensor_scalar(M4[:, 0:N], iota4[:, 0:N], delta, cyx[:, 0:1],
                                ALU.mult, ALU.add)
        nc.vector.tensor_scalar(M4[:, N:2 * N], iota4[:, N:2 * N], delta,
                                cyx[:, 1:2], ALU.mult, ALU.add)
        nc.vector.tensor_scalar(M4[:, 2 * N:2 * N + 1], en, -0.5, None, ALU.mult)
        nc.vector.tensor_copy(M4[:, 2 * N + 1:2 * N + 2], E[:, 2:3])
        # all batches' rows flattened onto partition 0 (one sbuf->sbuf DMA);
        # partition_broadcast can only read from absolute partition 0.
        # Issued from the Act queue so it never delays the x-load stream
        # on the SP queue (it waits on M4 while holding its sequencer).
        stage = params.tile([1, BL, 2 * N + 2], F32)
        nc.scalar.dma_start(out=stage, in_=M4)

        # ---- all 12 x loads issued up front (keeps DMA queue full) -----
        xf_tiles = []
        for b in range(BL):
            row = []
            for c in range(C):
                xf = xf32p.tile([128, KC, W], F32)
                nc.sync.dma_start(
                    out=xf, in_=x_d[b, c].rearrange("(hc p) w -> p hc w", p=128))
                row.append(xf)
            xf_tiles.append(row)

        # ---- PE p-state primers (run while first tiles load) -----------
        for i in range(NPRIME):
            pp = ps2.tile([128, N], F32, tag="p2")
            nc.tensor.matmul(pp, czero[:, 0:128], czero[:, 0:N],
                             start=True, stop=True)

        # ---- all mu/nhs broadcasts upfront on Pool (depend only on stage,
        # so Pool never blocks a later batch's filterbank behind a conv) ---
        bcmus, nhs_gams = [], []
        for b in range(BL):
            bcmu = bcp.tile([128, 2 * N], F32)
            nc.gpsimd.partition_broadcast(bcmu, stage[0:1, b, 0:2 * N])
            nhs_gam = colp.tile([128, 2], F32)
            nc.gpsimd.partition_broadcast(nhs_gam, stage[0:1, b, 2 * N:2 * N + 2])
            bcmus.append(bcmu)
            nhs_gams.append(nhs_gam)

        for b in range(BL):
            bcmu, nhs_gam = bcmus[b], nhs_gams[b]
            # ---- filterbanks: Ty|Tx fused per chunk, unnormalized ------
            T = []
            cs_ps = pscs.tile([1, 2 * N], F32)
            for k in range(KC):
                sq_t = sqtmp.tile([128, 2 * N], F32)
                nc.scalar.activation(sq_t, bcmu, ACTF.Square,
                                     bias=nbias[:, k:k + 1])
                T_t = tban.tile([128, 2 * N], F16)
                nc.scalar.activation(T_t, sq_t, ACTF.Exp,
                                     scale=nhs_gam[:, 0:1])
                T.append(T_t)
                nc.tensor.matmul(cs_ps, ones_k, T_t,
                                 start=(k == 0), stop=(k == KC - 1))
            # 1/(colsum + SMALL) = exp(-ln(colsum + SMALL)), both axes at once
            lnrow = rows.tile([1, 2 * N], F32)
            nc.scalar.activation(lnrow, cs_ps, ACTF.Ln, bias=small1[:, :])
            invrow = rows.tile([1, 2 * N], F32)
            nc.scalar.activation(invrow, lnrow, ACTF.Exp, scale=-1.0)

            # fp32->fp16 conversions: c0/c1 on DVE, c2 on Pool (Act must
            # stay free for the next batch's filterbank)
            xt_row = []
            for c in range(C):
                xt = xbfp.tile([128, KC, W], F16)
                (nc.gpsimd if c == 2 else nc.vector).tensor_copy(
                    xt, xf_tiles[b][c])
                xt_row.append(xt)
            # x-normalizer broadcast across partitions (m on free axis)
            invx_bc = invp.tile([128, N], F32)
            nc.gpsimd.partition_broadcast(invx_bc, invrow[0:1, N:2 * N])

            # ---- glimpse read: two chained GEMMs per channel -----------
            for c in range(C):
                xt = xt_row[c]
                fyx = []
                for j in range(2):  # wc pairs
                    p1 = ps1.tile([128, 2 * N], F32)
                    for half in range(2):
                        wc = 2 * j + half
                        for hc in range(KC):
                            nc.tensor.matmul(
                                p1[:, half * N:(half + 1) * N],
                                xt[:, hc, wc * 128:(wc + 1) * 128],
                                T[hc][:, 0:N],
                                start=(hc == 0), stop=(hc == KC - 1))
                    f_t = fyxp.tile([128, 2 * N], F16)
                    nc.vector.tensor_copy(f_t, p1)
                    fyx.append(f_t)
                if c == 0:
                    # y-normalizer to column layout (n on partitions), *gamma.
                    # Emitted after G1(c0) so the tiny transpose matmuls
                    # never gate the GEMM stream on the Act ln/exp chain.
                    invy_col = colp.tile([128, 2], F32)
                    pcol2 = pscol.tile([128, 2], F32)
                    for j in range(2):
                        nc.tensor.matmul(pcol2[:, j:j + 1],
                                         invrow[:, j * 128:(j + 1) * 128],
                                         one1, start=True, stop=True)
                    nc.vector.tensor_scalar(invy_col, pcol2,
                                            nhs_gam[:, 1:2], None, ALU.mult)
                ot = outp.tile([128, 2, N], F16)
                for nch in range(2):
                    p2 = ps2.tile([128, N], F32, tag="p2")
                    for wc in range(KC):
                        nc.tensor.matmul(
                            p2,
                            fyx[wc // 2][:, (wc % 2) * N + nch * 128:
                                         (wc % 2) * N + (nch + 1) * 128],
                            T[wc][:, N:2 * N],
                            start=(wc == 0), stop=(wc == KC - 1))
                    nc.vector.scalar_tensor_tensor(ot[:, nch, :], p2,
                                                   invy_col[:, nch:nch + 1],
                                                   invx_bc, ALU.mult, ALU.mult)
                nc.sync.dma_start(
                    out=o_d[b, c].rearrange("(nch p) m -> p nch m", p=128), in_=ot)


_NC_CACHE = None


def _build():
    global _NC_CACHE
    if _NC_CACHE is None:
        nc = bacc.Bacc("TRN2", target_bir_lowering=False, debug=False,
                       enable_asserts=False, num_devices=NCORES)
        with tile.TileContext(nc) as tc:
            _kernel_body(tc)
        # Steer bacc's greedy ACT table-set choice to the one set that has
        # Exp+Ln+Square+Copy+Identity, else every per-batch Ln costs two
        # ~2.7us table reloads. Only the selection input is patched — set
        # ids and on-device table contents are untouched.
        ours = {ACTF.Exp, ACTF.Ln, ACTF.Square, ACTF.Copy, ACTF.Identity}
        keep = "natural_log_exp_and_others"
        orig = bacc.get_activation_tables

        def steered(arch):
            return {k: (v if k == keep else set(v) - ours)
                    for k, v in orig(arch).items()}

        bacc.get_activation_tables = steered
        try:
            nc.compile()
        finally:
            bacc.get_activation_tables = orig
        _NC_CACHE = nc
    return _NC_CACHE


def _run(x, p, trace=False, **kw):
    nc = _build()
    x = np.ascontiguousarray(x, dtype=np.float32)
    p = np.ascontiguousarray(p, dtype=np.float32)
    assert x.shape == (B, C, H, W) and p.shape == (B, 5), (x.shape, p.shape)
    in_maps = [
        {"x": x[i * BL:(i + 1) * BL], "p": p[i * BL:(i + 1) * BL]}
        for i in range(NCORES)
    ]
    res = run_bass_kernel_spmd(nc, in_maps, list(range(NCORES)), trace=trace, **kw)
    out = np.concatenate(
        [np.asarray(res.results[i]["out"], dtype=np.float32) for i in range(NCORES)],
        axis=0)
    return out, res


def kernel(x, p):
    out, _ = _run(x, p)
    return out


# revision 17
# speedup vs baseline: 1.0107x; 1.0107x over previous
"""DifferentiableRAM (DRAW-style attention read) Trainium2 Bass kernel.

Reference computation (per batch b, channel c):
    gx = W*(p0+1)/2, gy = H*(p1+1)/2, sigma2 = exp(p2),
    delta = exp(p3)*(W-1)/(N-1), gamma = exp(p4)
    mu[i]  = g + delta*(i - N/2 - 0.5)                      i in [0,N)
    F[i,a] = exp(-(a-mu[i])^2 / (2 sigma2)) ;  Fn = F / (F.sum(a) + 1e-4)
    out[b,c] = gamma * Fy_n @ x[b,c] @ Fx_n^T                [N, N]

Strategy: pure data parallel over batch (B=32 -> 4 per core on 8 cores).
Both filterbanks are generated transposed, T[a, i] = exp(nhs*(a-mu_i)^2),
y and x halves side by side in one [128, 512] tile per 128-row chunk, so
both GEMMs contract along the partition axis:
    G1: FyxT[w, n] = sum_h x[h, w] * Ty[h, n]      (lhsT = x chunk)
    G2: raw[n, m]  = sum_w FyxT[w, n] * Tx[w, m]   (lhsT = FyxT chunk)
    out[n, m] = raw[n, m] * (gamma * invy[n]) * invx[m]
GEMMs run in fp16 (full PE rate).  Normalizers stay fp32 and are applied
to the fp32 PSUM of G2.

Performance structure (driven by the TimelineSim cost model):
  - one big DMA per (b,c) x tile and per (b,c) output tile (25 DMAs
    total) to keep the SP sequencer + HWDGE dispatch path off the
    critical path; all 12 x loads are issued up front so DMA streams
    back-to-back,
  - output stored as fp16 (host casts back to fp32) to halve store
    traffic,
  - mu rows broadcast to all partitions with gpsimd partition_broadcast
    (idle Pool engine) instead of fp32 PE matmuls,
  - d = mu - a is folded into the Square activation via a per-partition
    bias column, T = Exp(nhs * Square(mu + bias)),
  - PSUM->SBUF copies on Pool, fp32->fp16 input conversion split across
    DVE/Act/Pool, output scaling on DVE,
  - a few zero-weight matmuls prime the PE p-state ramp while the first
    tiles load.
"""

import numpy as np
from contextlib import ExitStack

import concourse.tile as tile
from concourse import bacc, mybir
from concourse.bass_utils import run_bass_kernel_spmd

F32 = mybir.dt.float32
F16 = mybir.dt.float16
ALU = mybir.AluOpType
ACTF = mybir.ActivationFunctionType

B, C, H, W = 32, 3, 512, 512
N = 256
NCORES = 8
BL = B // NCORES  # batches per core
KC = 4            # 128-row chunks of the 512-long axis
SMALL = 1e-4
DELTA_SCALE = (max(W, H) - 1) / (N - 1.0)
NPRIME = 16       # PE p-state priming matmuls


def _kernel_body(tc):
    nc = tc.nc
    x_d = nc.dram_tensor("x", [BL, C, H, W], F32, kind="ExternalInput").ap()
    p_d = nc.dram_tensor("p", [BL, 5], F32, kind="ExternalInput").ap()
    o_d = nc.dram_tensor("out", [BL, C, N, N], F16, kind="ExternalOutput").ap()

    with ExitStack() as ctx:
        consts = ctx.enter_context(tc.tile_pool(name="consts", bufs=1))
        params = ctx.enter_context(tc.tile_pool(name="params", bufs=1))
        xf32p = ctx.enter_context(tc.tile_pool(name="xf32p", bufs=12))
        xbfp = ctx.enter_context(tc.tile_pool(name="xbfp", bufs=6))
        tban = ctx.enter_context(tc.tile_pool(name="tban", bufs=8))
        bcp = ctx.enter_context(tc.tile_pool(name="bcp", bufs=4))
        sqtmp = ctx.enter_context(tc.tile_pool(name="sqtmp", bufs=3))
        fyxp = ctx.enter_context(tc.tile_pool(name="fyxp", bufs=4))
        outp = ctx.enter_context(tc.tile_pool(name="outp", bufs=3))
        rows = ctx.enter_context(tc.tile_pool(name="rows", bufs=4))
        colp = ctx.enter_context(tc.tile_pool(name="colp", bufs=4))
        invp = ctx.enter_context(tc.tile_pool(name="invp", bufs=2))
        # PSUM: 8 banks - ps1 4 + ps2 2 + pscs 1 + pscol 1
        ps1 = ctx.enter_context(tc.tile_pool(name="ps1", bufs=4, space="PSUM"))
        ps2 = ctx.enter_context(tc.tile_pool(name="ps2", bufs=2, space="PSUM"))
        pscs = ctx.enter_context(tc.tile_pool(name="pscs", bufs=1, space="PSUM"))
        pscol = ctx.enter_context(tc.tile_pool(name="pscol", bufs=1, space="PSUM"))

        # ---- constants -------------------------------------------------
        # nbias[p, k] = -(p + 128k): per-partition bias for the Square
        nbias = consts.tile([128, KC], F32)
        nc.gpsimd.iota(nbias, pattern=[[-128, KC]], base=0,
                       channel_multiplier=-1,
                       allow_small_or_imprecise_dtypes=True)
        iota2 = consts.tile([1, 2 * N], F32)  # 0..255 twice, on partition 0
        nc.gpsimd.iota(iota2, pattern=[[0, 2], [1, N]], base=0,
                       channel_multiplier=0, allow_small_or_imprecise_dtypes=True)
        ones_k = consts.tile([128, 1], F16)  # colsum lhsT
        nc.vector.memset(ones_k, 1.0)
        one1 = consts.tile([1, 1], F32)       # row->col rhs
        nc.vector.memset(one1, 1.0)
        small1 = consts.tile([1, 1], F32)     # filterbank-normalizer epsilon
        nc.vector.memset(small1, SMALL)
        czero = consts.tile([128, 2 * N], F16)  # PE p-state primer operand
        nc.vector.memset(czero, 0.0)

        # ---- attention params, all on partition 0 ----------------------
        # p is loaded straight into a partition-0 row (the load DMA does
        # the [4,5] -> [1,4,5] flatten) because partition_broadcast can
        # only read from absolute partition 0; computing the mu rows there
        # avoids an sbuf->sbuf staging DMA that would queue behind the
        # multi-us x-load transfers on the serialized DMA engines.
        pt0 = params.tile([1, BL, 5], F32)
        nc.sync.dma_start(out=pt0, in_=p_d)
        E0 = params.tile([1, BL, 3], F32)  # [sigma2, exp(p3), gamma]
        nc.scalar.activation(E0, pt0[0:1, :, 2:5], ACTF.Exp)
        en0 = params.tile([1, BL, 1], F32)  # exp(-p2) = 1/sigma2
        nc.scalar.activation(en0, pt0[0:1, :, 2:3], ACTF.Exp, scale=-1.0)
        sc = params.tile([1, BL, 5], F32)  # [delta, gy, gx, cy, cx]
        nc.vector.tensor_scalar(sc[0:1, :, 0:1], E0[0:1, :, 1:2], DELTA_SCALE,
                                None, ALU.mult)
        nc.vector.tensor_scalar(sc[0:1, :, 1:2], pt0[0:1, :, 1:2], W / 2.0,
                                W / 2.0, ALU.mult, ALU.add)
        nc.vector.tensor_scalar(sc[0:1, :, 2:3], pt0[0:1, :, 0:1], W / 2.0,
                                W / 2.0, ALU.mult, ALU.add)
        nc.vector.scalar_tensor_tensor(sc[0:1, :, 3:4], sc[0:1, :, 0:1],
                                       -(N / 2.0 + 0.5), sc[0:1, :, 1:2],
                                       ALU.mult, ALU.add)
        nc.vector.scalar_tensor_tensor(sc[0:1, :, 4:5], sc[0:1, :, 0:1],
                                       -(N / 2.0 + 0.5), sc[0:1, :, 2:3],
                                       ALU.mult, ALU.add)
        # per-batch row: [mu_y (N) | mu_x (N) | nhs | gamma] on partition 0
        stage = params.tile([1, BL, 2 * N + 2], F32)
        for b in range(BL):
            nc.vector.tensor_scalar(stage[0:1, b, 0:N], iota2[0:1, 0:N],
                                    sc[0:1, b, 0:1], sc[0:1, b, 3:4],
                                    ALU.mult, ALU.add)
            nc.vector.tensor_scalar(stage[0:1, b, N:2 * N], iota2[0:1, N:2 * N],
                                    sc[0:1, b, 0:1], sc[0:1, b, 4:5],
                                    ALU.mult, ALU.add)
        nc.vector.tensor_scalar(stage[0:1, :, 2 * N:2 * N + 1], en0, -0.5,
                                None, ALU.mult)
        nc.vector.tensor_copy(stage[0:1, :, 2 * N + 1:2 * N + 2],
                              E0[0:1, :, 2:3])

        # ---- all 12 x loads issued up front (keeps DMA queue full) -----
        xf_tiles = []
        for b in range(BL):
            row = []
            for c in range(C):
                xf = xf32p.tile([128, KC, W], F32)
                nc.sync.dma_start(
                    out=xf, in_=x_d[b, c].rearrange("(hc p) w -> p hc w", p=128))
                row.append(xf)
            xf_tiles.append(row)

        # ---- PE p-state primers (run while first tiles load) -----------
        for i in range(NPRIME):
            pp = ps2.tile([128, N], F32, tag="p2")
            nc.tensor.matmul(pp, czero[:, 0:128], czero[:, 0:N],
                             start=True, stop=True)

        # ---- all mu/nhs broadcasts upfront on Pool (depend only on stage,
        # so Pool never blocks a later batch's filterbank behind a conv) ---
        bcmus, nhs_gams = [], []
        for b in range(BL):
            bcmu = bcp.tile([128, 2 * N], F32)
            nc.gpsimd.partition_broadcast(bcmu, stage[0:1, b, 0:2 * N])
            nhs_gam = colp.tile([128, 2], F32)
            nc.gpsimd.partition_broadcast(nhs_gam, stage[0:1, b, 2 * N:2 * N + 2])
            bcmus.append(bcmu)
            nhs_gams.append(nhs_gam)

        for b in range(BL):
            bcmu, nhs_gam = bcmus[b], nhs_gams[b]
            # ---- filterbanks: Ty|Tx fused per chunk, unnormalized ------
            T = []
            cs_ps = pscs.tile([1, 2 * N], F32)
            for k in range(KC):
                sq_t = sqtmp.tile([128, 2 * N], F32)
                nc.scalar.activation(sq_t, bcmu, ACTF.Square,
                                     bias=nbias[:, k:k + 1])
                T_t = tban.tile([128, 2 * N], F16)
                nc.scalar.activation(T_t, sq_t, ACTF.Exp,
                                     scale=nhs_gam[:, 0:1])
                T.append(T_t)
                nc.tensor.matmul(cs_ps, ones_k, T_t,
                                 start=(k == 0), stop=(k == KC - 1))
            # 1/(colsum + SMALL) = exp(-ln(colsum + SMALL)), both axes at once
            lnrow = rows.tile([1, 2 * N], F32)
            nc.scalar.activation(lnrow, cs_ps, ACTF.Ln, bias=small1[:, :])
            invrow = rows.tile([1, 2 * N], F32)
            nc.scalar.activation(invrow, lnrow, ACTF.Exp, scale=-1.0)

            # fp32->fp16 conversions: c0/c1 on DVE, c2 on Pool (Act must
            # stay free for the next batch's filterbank)
            xt_row = []
            for c in range(C):
                xt = xbfp.tile([128, KC, W], F16)
                (nc.gpsimd if c == 2 else nc.vector).tensor_copy(
                    xt, xf_tiles[b][c])
                xt_row.append(xt)
            # x-normalizer broadcast across partitions (m on free axis)
            invx_bc = invp.tile([128, N], F32)
            nc.gpsimd.partition_broadcast(invx_bc, invrow[0:1, N:2 * N])

            # ---- glimpse read: two chained GEMMs per channel -----------
            for c in range(C):
                xt = xt_row[c]
                fyx = []
                for j in range(2):  # wc pairs
                    p1 = ps1.tile([128, 2 * N], F32)
                    for half in range(2):
                        wc = 2 * j + half
                        for hc in range(KC):
                            nc.tensor.matmul(
                                p1[:, half * N:(half + 1) * N],
                                xt[:, hc, wc * 128:(wc + 1) * 128],
                                T[hc][:, 0:N],
                                start=(hc == 0), stop=(hc == KC - 1))
                    f_t = fyxp.tile([128, 2 * N], F16)
                    nc.vector.tensor_copy(f_t, p1)
                    fyx.append(f_t)
                if c == 0:
                    # y-normalizer to column layout (n on partitions), *gamma.
                    # Emitted after G1(c0) so the tiny transpose matmuls
                    # never gate the GEMM stream on the Act ln/exp chain.
                    invy_col = colp.tile([128, 2], F32)
                    pcol2 = pscol.tile([128, 2], F32)
                    for j in range(2):
                        nc.tensor.matmul(pcol2[:, j:j + 1],
                                         invrow[:, j * 128:(j + 1) * 128],
                                         one1, start=True, stop=True)
                    nc.vector.tensor_scalar(invy_col, pcol2,
                                            nhs_gam[:, 1:2], None, ALU.mult)
                ot = outp.tile([128, 2, N], F16)
                for nch in range(2):
                    p2 = ps2.tile([128, N], F32, tag="p2")
                    for wc in range(KC):
                        nc.tensor.matmul(
                            p2,
                            fyx[wc // 2][:, (wc % 2) * N + nch * 128:
                                         (wc % 2) * N + (nch + 1) * 128],
                            T[wc][:, N:2 * N],
                            start=(wc == 0), stop=(wc == KC - 1))
                    nc.vector.scalar_tensor_tensor(ot[:, nch, :], p2,
                                                   invy_col[:, nch:nch + 1],
                                                   invx_bc, ALU.mult, ALU.mult)
                nc.sync.dma_start(
                    out=o_d[b, c].rearrange("(nch p) m -> p nch m", p=128), in_=ot)


_NC_CACHE = None


def _build():
    global _NC_CACHE
    if _NC_CACHE is None:
        nc = bacc.Bacc("TRN2", target_bir_lowering=False, debug=False,
                       enable_asserts=False, num_devices=NCORES)
        with tile.TileContext(nc) as tc:
            _kernel_body(tc)
        # Steer bacc's greedy ACT table-set choice to the one set that has
        # Exp+Ln+Square+Copy+Identity, else every per-batch Ln costs two
        # ~2.7us table reloads. Only the selection input is patched — set
        # ids and on-device table contents are untouched.
        ours = {ACTF.Exp, ACTF.Ln, ACTF.Square, ACTF.Copy, ACTF.Identity}
        keep = "natural_log_exp_and_others"
        orig = bacc.get_activation_tables

        def steered(arch):
            return {k: (v if k == keep else set(v) - ours)
                    for k, v in orig(arch).items()}

        bacc.get_activation_tables = steered
        try:
            nc.compile()
        finally:
            bacc.get_activation_tables = orig
        _NC_CACHE = nc
    return _NC_CACHE


def _run(x, p, trace=False, **kw):
    nc = _build()
    x = np.ascontiguousarray(x, dtype=np.float32)
    p = np.ascontiguousarray(p, dtype=np.float32)
    assert x.shape == (B, C, H, W) and p.shape == (B, 5), (x.shape, p.shape)
    in_maps = [
        {"x": x[i * BL:(i + 1) * BL], "p": p[i * BL:(i + 1) * BL]}
        for i in range(NCORES)
    ]
    res = run_bass_kernel_spmd(nc, in_maps, list(range(NCORES)), trace=trace, **kw)
    out = np.concatenate(
        [np.asarray(res.results[i]["out"], dtype=np.float32) for i in range(NCORES)],
        axis=0)
    return out, res


def kernel(x, p):
    out, _ = _run(x, p)
    return out


# revision 22
# speedup vs baseline: 1.1367x; 1.1246x over previous
"""DifferentiableRAM (DRAW-style attention read) Trainium2 Bass kernel.

Reference computation (per batch b, channel c):
    gx = W*(p0+1)/2, gy = H*(p1+1)/2, sigma2 = exp(p2),
    delta = exp(p3)*(W-1)/(N-1), gamma = exp(p4)
    mu[i]  = g + delta*(i - N/2 - 0.5)                      i in [0,N)
    F[i,a] = exp(-(a-mu[i])^2 / (2 sigma2)) ;  Fn = F / (F.sum(a) + 1e-4)
    out[b,c] = gamma * Fy_n @ x[b,c] @ Fx_n^T                [N, N]

Strategy: pure data parallel over batch (B=32 -> 4 per core on 8 cores).
Both filterbanks are generated transposed, T[a, i] = exp(nhs*(a-mu_i)^2),
y and x halves side by side in one [128, 512] tile per 128-row chunk, so
both GEMMs contract along the partition axis:
    G1: FyxT[w, n] = sum_h x[h, w] * Ty[h, n]      (lhsT = x chunk)
    G2: raw[n, m]  = sum_w FyxT[w, n] * Tx[w, m]   (lhsT = FyxT chunk)
    out[n, m] = raw[n, m] * (gamma * invy[n]) * invx[m]
GEMMs run in fp16 (full PE rate).  Normalizers stay fp32 and are applied
to the fp32 PSUM of G2.

Performance structure (driven by the TimelineSim cost model):
  - one big DMA per (b,c) x tile and per (b,c) output tile (25 DMAs
    total) to keep the SP sequencer + HWDGE dispatch path off the
    critical path; all 12 x loads are issued up front so DMA streams
    back-to-back,
  - output stored as fp16 (host casts back to fp32) to halve store
    traffic,
  - mu rows broadcast to all partitions with gpsimd partition_broadcast
    (idle Pool engine) instead of fp32 PE matmuls,
  - d = mu - a is folded into the Square activation via a per-partition
    bias column, T = Exp(nhs * Square(mu + bias)),
  - PSUM->SBUF copies on Pool, fp32->fp16 input conversion split across
    DVE/Act/Pool, output scaling on DVE,
  - a few zero-weight matmuls prime the PE p-state ramp while the first
    tiles load.
"""

import numpy as np
from contextlib import ExitStack

import concourse.tile as tile
from concourse import bacc, mybir
from concourse.bass_utils import run_bass_kernel_spmd

F32 = mybir.dt.float32
F16 = mybir.dt.float16
ALU = mybir.AluOpType
ACTF = mybir.ActivationFunctionType

B, C, H, W = 32, 3, 512, 512
N = 256
NCORES = 8
BL = B // NCORES  # batches per core
KC = 4            # 128-row chunks of the 512-long axis
SMALL = 1e-4
DELTA_SCALE = (max(W, H) - 1) / (N - 1.0)
NPRIME = 16       # PE p-state priming matmuls


def _kernel_body(tc):
    nc = tc.nc
    x_d = nc.dram_tensor("x", [BL, C, H, W], F32, kind="ExternalInput").ap()
    p_d = nc.dram_tensor("p", [BL, 5], F32, kind="ExternalInput").ap()
    o_d = nc.dram_tensor("out", [BL, C, N, N], F16, kind="ExternalOutput").ap()

    with ExitStack() as ctx:
        consts = ctx.enter_context(tc.tile_pool(name="consts", bufs=1))
        params = ctx.enter_context(tc.tile_pool(name="params", bufs=1))
        xf32p = ctx.enter_context(tc.tile_pool(name="xf32p", bufs=12))
        xbfp = ctx.enter_context(tc.tile_pool(name="xbfp", bufs=6))
        tban = ctx.enter_context(tc.tile_pool(name="tban", bufs=8))
        bcp = ctx.enter_context(tc.tile_pool(name="bcp", bufs=4))
        sqtmp = ctx.enter_context(tc.tile_pool(name="sqtmp", bufs=3))
        fyxp = ctx.enter_context(tc.tile_pool(name="fyxp", bufs=4))
        outp = ctx.enter_context(tc.tile_pool(name="outp", bufs=3))
        rows = ctx.enter_context(tc.tile_pool(name="rows", bufs=4))
        colp = ctx.enter_context(tc.tile_pool(name="colp", bufs=4))
        invp = ctx.enter_context(tc.tile_pool(name="invp", bufs=2))
        # PSUM: 8 banks - ps1 4 + ps2 3 + pscs 1
        ps1 = ctx.enter_context(tc.tile_pool(name="ps1", bufs=4, space="PSUM"))
        ps2 = ctx.enter_context(tc.tile_pool(name="ps2", bufs=3, space="PSUM"))
        pscs = ctx.enter_context(tc.tile_pool(name="pscs", bufs=1, space="PSUM"))

        # ---- constants -------------------------------------------------
        # nbias[p, k] = -(p + 128k): per-partition bias for the Square
        nbias = consts.tile([128, KC], F32)
        nc.gpsimd.iota(nbias, pattern=[[-128, KC]], base=0,
                       channel_multiplier=-1,
                       allow_small_or_imprecise_dtypes=True)
        iota2 = consts.tile([1, 2 * N], F32)  # 0..255 twice, on partition 0
        nc.gpsimd.iota(iota2, pattern=[[0, 2], [1, N]], base=0,
                       channel_multiplier=0, allow_small_or_imprecise_dtypes=True)
        ones_k = consts.tile([128, 1], F16)  # colsum lhsT
        nc.vector.memset(ones_k, 1.0)
        small1 = consts.tile([1, 1], F32)     # filterbank-normalizer epsilon
        nc.vector.memset(small1, SMALL)
        czero = consts.tile([128, 2 * N], F16)  # PE p-state primer operand
        nc.vector.memset(czero, 0.0)

        # ---- attention params, all on partition 0 ----------------------
        # p is loaded straight into a partition-0 row (the load DMA does
        # the [4,5] -> [1,4,5] flatten) because partition_broadcast can
        # only read from absolute partition 0; computing the mu rows there
        # avoids an sbuf->sbuf staging DMA that would queue behind the
        # multi-us x-load transfers on the serialized DMA engines.
        pt0 = params.tile([1, BL, 5], F32)
        nc.sync.dma_start(out=pt0, in_=p_d)
        E0 = params.tile([1, BL, 3], F32)  # [sigma2, exp(p3), gamma]
        nc.scalar.activation(E0, pt0[0:1, :, 2:5], ACTF.Exp)
        en0 = params.tile([1, BL, 1], F32)  # exp(-p2) = 1/sigma2
        nc.scalar.activation(en0, pt0[0:1, :, 2:3], ACTF.Exp, scale=-1.0)
        sc = params.tile([1, BL, 5], F32)  # [delta, gy, gx, cy, cx]
        nc.vector.tensor_scalar(sc[0:1, :, 0:1], E0[0:1, :, 1:2], DELTA_SCALE,
                                None, ALU.mult)
        nc.vector.tensor_scalar(sc[0:1, :, 1:2], pt0[0:1, :, 1:2], W / 2.0,
                                W / 2.0, ALU.mult, ALU.add)
        nc.vector.tensor_scalar(sc[0:1, :, 2:3], pt0[0:1, :, 0:1], W / 2.0,
                                W / 2.0, ALU.mult, ALU.add)
        nc.vector.scalar_tensor_tensor(sc[0:1, :, 3:4], sc[0:1, :, 0:1],
                                       -(N / 2.0 + 0.5), sc[0:1, :, 1:2],
                                       ALU.mult, ALU.add)
        nc.vector.scalar_tensor_tensor(sc[0:1, :, 4:5], sc[0:1, :, 0:1],
                                       -(N / 2.0 + 0.5), sc[0:1, :, 2:3],
                                       ALU.mult, ALU.add)
        # per-batch row: [mu_y (N) | mu_x (N) | nhs] on partition 0
        stage = params.tile([1, BL, 2 * N + 1], F32)
        for b in range(BL):
            nc.vector.tensor_scalar(stage[0:1, b, 0:N], iota2[0:1, 0:N],
                                    sc[0:1, b, 0:1], sc[0:1, b, 3:4],
                                    ALU.mult, ALU.add)
            nc.vector.tensor_scalar(stage[0:1, b, N:2 * N], iota2[0:1, N:2 * N],
                                    sc[0:1, b, 0:1], sc[0:1, b, 4:5],
                                    ALU.mult, ALU.add)
        nc.vector.tensor_scalar(stage[0:1, :, 2 * N:2 * N + 1], en0, -0.5,
                                None, ALU.mult)

        # ---- all 12 x loads issued up front (keeps DMA queue full) -----
        xf_tiles = []
        for b in range(BL):
            row = []
            for c in range(C):
                xf = xf32p.tile([128, KC, W], F32)
                nc.sync.dma_start(
                    out=xf, in_=x_d[b, c].rearrange("(hc p) w -> p hc w", p=128))
                row.append(xf)
            xf_tiles.append(row)

        # ---- PE p-state primers (run while first tiles load) -----------
        for i in range(NPRIME):
            pp = ps2.tile([128, 2, N], F32, tag="p2")
            nc.tensor.matmul(pp[:, 0, :], czero[:, 0:128], czero[:, 0:N],
                             start=True, stop=True)

        # ---- all mu/nhs broadcasts upfront on Pool (depend only on stage,
        # so Pool never blocks a later batch's filterbank behind a conv) ---
        bcmus, nhs_cols = [], []
        for b in range(BL):
            bcmu = bcp.tile([128, 2 * N], F32)
            nc.gpsimd.partition_broadcast(bcmu, stage[0:1, b, 0:2 * N])
            nhs_col = colp.tile([128, 1], F32)
            nc.gpsimd.partition_broadcast(nhs_col, stage[0:1, b, 2 * N:2 * N + 1])
            bcmus.append(bcmu)
            nhs_cols.append(nhs_col)

        def load_ready_ms(b, c):
            # Estimated real arrival of x tile (b, c): the serialized DMA
            # engines deliver one 1MB tile every ~2.9us. Stamping the
            # conversions with this time keeps the Tile scheduler (whose
            # own DMA model is far more optimistic) from hoisting a
            # load-gated conversion ahead of urgent PE-coupled work.
            return (2.2 + 2.912 * (3 * b + c + 1)) / 1000.0

        for b in range(BL):
            bcmu, nhs_col = bcmus[b], nhs_cols[b]
            # ---- filterbanks: Ty|Tx fused per chunk, unnormalized ------
            T = []
            cs_ps = pscs.tile([1, 2 * N], F32)
            for k in range(KC):
                sq_t = sqtmp.tile([128, 2 * N], F32)
                nc.scalar.activation(sq_t, bcmu, ACTF.Square,
                                     bias=nbias[:, k:k + 1])
                T_t = tban.tile([128, 2 * N], F16)
                nc.scalar.activation(T_t, sq_t, ACTF.Exp,
                                     scale=nhs_col[:, 0:1])
                T.append(T_t)
                nc.tensor.matmul(cs_ps, ones_k, T_t,
                                 start=(k == 0), stop=(k == KC - 1))
            # normalizer rows on partition 0: ry = gamma/(csy+eps) via
            # exp(-ln(cs+eps) + p4)  (gamma = exp(p4), so ln gamma = p4),
            # rx = 1/(csx+eps)
            lnrow = rows.tile([1, 2 * N], F32)
            nc.scalar.activation(lnrow, cs_ps, ACTF.Ln, bias=small1[:, :])
            ry = rows.tile([1, N], F32)
            nc.scalar.activation(ry, lnrow[0:1, 0:N], ACTF.Exp, scale=-1.0,
                                 bias=pt0[0:1, b, 4:5])
            rx = rows.tile([1, N], F32)
            nc.scalar.activation(rx, lnrow[0:1, N:2 * N], ACTF.Exp, scale=-1.0)
            # broadcast both rows to [128, 2, N] (each half identical) so
            # they fold multiplicatively into the f_t copy / output scale
            invy2 = invp.tile([128, 2, N], F32)
            invx2 = invp.tile([128, 2, N], F32)
            for j in range(2):
                nc.gpsimd.partition_broadcast(invy2[:, j], ry)
                nc.gpsimd.partition_broadcast(invx2[:, j], rx)

            # fp32->fp16 conversions: c0/c1 on DVE, c2 on Act, each
            # stamped with its tile's estimated DMA arrival time
            xt_row = []
            for c in range(C):
                xt = xbfp.tile([128, KC, W], F16)
                with tc.tile_wait_until(load_ready_ms(b, c)):
                    if c == 2:
                        nc.scalar.copy(xt, xf_tiles[b][c])
                    else:
                        nc.vector.tensor_copy(xt, xf_tiles[b][c])
                xt_row.append(xt)

            # ---- glimpse read: two chained GEMMs per channel -----------
            for c in range(C):
                xt = xt_row[c]
                fyx = []
                for j in range(2):  # wc pairs
                    p1 = ps1.tile([128, 2 * N], F32)
                    for half in range(2):
                        wc = 2 * j + half
                        for hc in range(KC):
                            nc.tensor.matmul(
                                p1[:, half * N:(half + 1) * N],
                                xt[:, hc, wc * 128:(wc + 1) * 128],
                                T[hc][:, 0:N],
                                start=(hc == 0), stop=(hc == KC - 1))
                    # PSUM->SBUF with the y-normalizer (and gamma) folded in
                    f_t = fyxp.tile([128, 2 * N], F16)
                    nc.vector.tensor_tensor(
                        f_t, p1, invy2.rearrange("p a b -> p (a b)"), ALU.mult)
                    fyx.append(f_t)
                ot = outp.tile([128, 2, N], F16)
                p2 = ps2.tile([128, 2, N], F32, tag="p2")
                for nch in range(2):
                    for wc in range(KC):
                        nc.tensor.matmul(
                            p2[:, nch, :],
                            fyx[wc // 2][:, (wc % 2) * N + nch * 128:
                                         (wc % 2) * N + (nch + 1) * 128],
                            T[wc][:, N:2 * N],
                            start=(wc == 0), stop=(wc == KC - 1))
                # single fused output scale: *= invx (both nch halves)
                nc.vector.tensor_tensor(
                    ot.rearrange("p a b -> p (a b)"),
                    p2.rearrange("p a b -> p (a b)"),
                    invx2.rearrange("p a b -> p (a b)"), ALU.mult)
                nc.sync.dma_start(
                    out=o_d[b, c].rearrange("(nch p) m -> p nch m", p=128), in_=ot)


_NC_CACHE = None


def _build():
    global _NC_CACHE
    if _NC_CACHE is None:
        nc = bacc.Bacc("TRN2", target_bir_lowering=False, debug=False,
                       enable_asserts=False, num_devices=NCORES)
        with tile.TileContext(nc) as tc:
            _kernel_body(tc)
        # Steer bacc's greedy ACT table-set choice to the one set that has
        # Exp+Ln+Square+Copy+Identity, else every per-batch Ln costs two
        # ~2.7us table reloads. Only the selection input is patched — set
        # ids and on-device table contents are untouched.
        ours = {ACTF.Exp, ACTF.Ln, ACTF.Square, ACTF.Copy, ACTF.Identity}
        keep = "natural_log_exp_and_others"
        orig = bacc.get_activation_tables

        def steered(arch):
            return {k: (v if k == keep else set(v) - ours)
                    for k, v in orig(arch).items()}

        bacc.get_activation_tables = steered
        try:
            nc.compile()
        finally:
            bacc.get_activation_tables = orig
        _NC_CACHE = nc
    return _NC_CACHE


def _run(x, p, trace=False, **kw):
    nc = _build()
    x = np.ascontiguousarray(x, dtype=np.float32)
    p = np.ascontiguousarray(p, dtype=np.float32)
    assert x.shape == (B, C, H, W) and p.shape == (B, 5), (x.shape, p.shape)
    in_maps = [
        {"x": x[i * BL:(i + 1) * BL], "p": p[i * BL:(i + 1) * BL]}
        for i in range(NCORES)
    ]
    res = run_bass_kernel_spmd(nc, in_maps, list(range(NCORES)), trace=trace, **kw)
    out = np.concatenate(
        [np.asarray(res.results[i]["out"], dtype=np.float32) for i in range(NCORES)],
        axis=0)
    return out, res


def kernel(x, p):
    out, _ = _run(x, p)
    return out


# revision 25
# speedup vs baseline: 1.1370x; 1.0002x over previous
"""DifferentiableRAM (DRAW-style attention read) Trainium2 Bass kernel.

Reference computation (per batch b, channel c):
    gx = W*(p0+1)/2, gy = H*(p1+1)/2, sigma2 = exp(p2),
    delta = exp(p3)*(W-1)/(N-1), gamma = exp(p4)
    mu[i]  = g + delta*(i - N/2 - 0.5)                      i in [0,N)
    F[i,a] = exp(-(a-mu[i])^2 / (2 sigma2)) ;  Fn = F / (F.sum(a) + 1e-4)
    out[b,c] = gamma * Fy_n @ x[b,c] @ Fx_n^T                [N, N]

Strategy: pure data parallel over batch (B=32 -> 4 per core on 8 cores).
Both filterbanks are generated transposed, T[a, i] = exp(nhs*(a-mu_i)^2),
y and x halves side by side in one [128, 512] tile per 128-row chunk, so
both GEMMs contract along the partition axis:
    G1: FyxT[w, n] = sum_h x[h, w] * Ty[h, n]      (lhsT = x chunk)
    G2: raw[n, m]  = sum_w FyxT[w, n] * Tx[w, m]   (lhsT = FyxT chunk)
    out[n, m] = raw[n, m] * (gamma * invy[n]) * invx[m]
GEMMs run in fp16 (full PE rate).  Normalizers stay fp32 and are applied
to the fp32 PSUM of G2.

Performance structure (driven by the TimelineSim cost model):
  - one big DMA per (b,c) x tile and per (b,c) output tile (25 DMAs
    total) to keep the SP sequencer + HWDGE dispatch path off the
    critical path; all 12 x loads are issued up front so DMA streams
    back-to-back,
  - output stored as fp16 (host casts back to fp32) to halve store
    traffic,
  - mu rows broadcast to all partitions with gpsimd partition_broadcast
    (idle Pool engine) instead of fp32 PE matmuls,
  - d = mu - a is folded into the Square activation via a per-partition
    bias column, T = Exp(nhs * Square(mu + bias)),
  - PSUM->SBUF copies on Pool, fp32->fp16 input conversion split across
    DVE/Act/Pool, output scaling on DVE,
  - a few zero-weight matmuls prime the PE p-state ramp while the first
    tiles load.
"""

import numpy as np
from contextlib import ExitStack

import concourse.tile as tile
from concourse import bacc, mybir
from concourse.bass_utils import run_bass_kernel_spmd

F32 = mybir.dt.float32
F16 = mybir.dt.float16
ALU = mybir.AluOpType
ACTF = mybir.ActivationFunctionType

B, C, H, W = 32, 3, 512, 512
N = 256
NCORES = 8
BL = B // NCORES  # batches per core
KC = 4            # 128-row chunks of the 512-long axis
SMALL = 1e-4
DELTA_SCALE = (max(W, H) - 1) / (N - 1.0)
NPRIME = 16       # PE p-state priming matmuls


def _kernel_body(tc):
    nc = tc.nc
    x_d = nc.dram_tensor("x", [BL, C, H, W], F32, kind="ExternalInput").ap()
    p_d = nc.dram_tensor("p", [BL, 5], F32, kind="ExternalInput").ap()
    o_d = nc.dram_tensor("out", [BL, C, N, N], F16, kind="ExternalOutput").ap()

    with ExitStack() as ctx:
        consts = ctx.enter_context(tc.tile_pool(name="consts", bufs=1))
        params = ctx.enter_context(tc.tile_pool(name="params", bufs=1))
        xf32p = ctx.enter_context(tc.tile_pool(name="xf32p", bufs=12))
        xbfp = ctx.enter_context(tc.tile_pool(name="xbfp", bufs=6))
        tban = ctx.enter_context(tc.tile_pool(name="tban", bufs=8))
        bcp = ctx.enter_context(tc.tile_pool(name="bcp", bufs=4))
        sqtmp = ctx.enter_context(tc.tile_pool(name="sqtmp", bufs=3))
        fyxp = ctx.enter_context(tc.tile_pool(name="fyxp", bufs=4))
        outp = ctx.enter_context(tc.tile_pool(name="outp", bufs=3))
        rows = ctx.enter_context(tc.tile_pool(name="rows", bufs=4))
        colp = ctx.enter_context(tc.tile_pool(name="colp", bufs=4))
        invp = ctx.enter_context(tc.tile_pool(name="invp", bufs=2))
        # PSUM: 8 banks - ps1 4 + ps2 3 + pscs 1
        ps1 = ctx.enter_context(tc.tile_pool(name="ps1", bufs=4, space="PSUM"))
        ps2 = ctx.enter_context(tc.tile_pool(name="ps2", bufs=3, space="PSUM"))
        pscs = ctx.enter_context(tc.tile_pool(name="pscs", bufs=1, space="PSUM"))

        # ---- constants -------------------------------------------------
        # nbias[p, k] = -(p + 128k): per-partition bias for the Square
        nbias = consts.tile([128, KC], F32)
        nc.gpsimd.iota(nbias, pattern=[[-128, KC]], base=0,
                       channel_multiplier=-1,
                       allow_small_or_imprecise_dtypes=True)
        iota2 = consts.tile([1, 2 * N], F32)  # 0..255 twice, on partition 0
        nc.gpsimd.iota(iota2, pattern=[[0, 2], [1, N]], base=0,
                       channel_multiplier=0, allow_small_or_imprecise_dtypes=True)
        ones_k = consts.tile([128, 1], F16)  # colsum lhsT
        nc.vector.memset(ones_k, 1.0)
        small1 = consts.tile([1, 1], F32)     # filterbank-normalizer epsilon
        nc.vector.memset(small1, SMALL)
        czero = consts.tile([128, 2 * N], F16)  # PE p-state primer operand
        nc.vector.memset(czero, 0.0)

        # ---- attention params, all on partition 0 ----------------------
        # p is loaded straight into a partition-0 row (the load DMA does
        # the [4,5] -> [1,4,5] flatten) because partition_broadcast can
        # only read from absolute partition 0; computing the mu rows there
        # avoids an sbuf->sbuf staging DMA that would queue behind the
        # multi-us x-load transfers on the serialized DMA engines.
        pt0 = params.tile([1, BL, 5], F32)
        nc.sync.dma_start(out=pt0, in_=p_d)
        E0 = params.tile([1, BL, 3], F32)  # [sigma2, exp(p3), gamma]
        nc.scalar.activation(E0, pt0[0:1, :, 2:5], ACTF.Exp)
        en0 = params.tile([1, BL, 1], F32)  # exp(-p2) = 1/sigma2
        nc.scalar.activation(en0, pt0[0:1, :, 2:3], ACTF.Exp, scale=-1.0)
        sc = params.tile([1, BL, 5], F32)  # [delta, gy, gx, cy, cx]
        nc.vector.tensor_scalar(sc[0:1, :, 0:1], E0[0:1, :, 1:2], DELTA_SCALE,
                                None, ALU.mult)
        nc.vector.tensor_scalar(sc[0:1, :, 1:2], pt0[0:1, :, 1:2], W / 2.0,
                                W / 2.0, ALU.mult, ALU.add)
        nc.vector.tensor_scalar(sc[0:1, :, 2:3], pt0[0:1, :, 0:1], W / 2.0,
                                W / 2.0, ALU.mult, ALU.add)
        nc.vector.scalar_tensor_tensor(sc[0:1, :, 3:4], sc[0:1, :, 0:1],
                                       -(N / 2.0 + 0.5), sc[0:1, :, 1:2],
                                       ALU.mult, ALU.add)
        nc.vector.scalar_tensor_tensor(sc[0:1, :, 4:5], sc[0:1, :, 0:1],
                                       -(N / 2.0 + 0.5), sc[0:1, :, 2:3],
                                       ALU.mult, ALU.add)
        # per-batch row: [mu_y (N) | mu_x (N) | nhs] on partition 0
        stage = params.tile([1, BL, 2 * N + 1], F32)
        for b in range(BL):
            nc.vector.tensor_scalar(stage[0:1, b, 0:N], iota2[0:1, 0:N],
                                    sc[0:1, b, 0:1], sc[0:1, b, 3:4],
                                    ALU.mult, ALU.add)
            nc.vector.tensor_scalar(stage[0:1, b, N:2 * N], iota2[0:1, N:2 * N],
                                    sc[0:1, b, 0:1], sc[0:1, b, 4:5],
                                    ALU.mult, ALU.add)
        nc.vector.tensor_scalar(stage[0:1, :, 2 * N:2 * N + 1], en0, -0.5,
                                None, ALU.mult)

        # ---- all x loads issued up front (keeps DMA queue full). Each
        # (b,c) tile is split into two half-tile DMAs so conversion and
        # G1 can start on the first half while the second streams in.
        xf_tiles = []
        for b in range(BL):
            row = []
            for c in range(C):
                xf = xf32p.tile([128, KC, W], F32)
                xsrc = x_d[b, c].rearrange("(hc p) w -> p hc w", p=128)
                nc.sync.dma_start(out=xf[:, 0:2], in_=xsrc[:, 0:2])
                nc.sync.dma_start(out=xf[:, 2:4], in_=xsrc[:, 2:4])
                row.append(xf)
            xf_tiles.append(row)

        # ---- PE p-state primers (run while first tiles load) -----------
        for i in range(NPRIME):
            pp = ps2.tile([128, 2, N], F32, tag="p2")
            nc.tensor.matmul(pp[:, 0, :], czero[:, 0:128], czero[:, 0:N],
                             start=True, stop=True)

        # ---- all mu/nhs broadcasts upfront on Pool (depend only on stage,
        # so Pool never blocks a later batch's filterbank behind a conv) ---
        bcmus, nhs_cols = [], []
        for b in range(BL):
            bcmu = bcp.tile([128, 2 * N], F32)
            nc.gpsimd.partition_broadcast(bcmu, stage[0:1, b, 0:2 * N])
            nhs_col = colp.tile([128, 1], F32)
            nc.gpsimd.partition_broadcast(nhs_col, stage[0:1, b, 2 * N:2 * N + 1])
            bcmus.append(bcmu)
            nhs_cols.append(nhs_col)

        def load_ready_ms(b, c, half):
            # Estimated real arrival of x half-tile (b, c, half): the
            # serialized DMA engines deliver one 512KB half every ~1.46us.
            # Stamping the conversions with this time keeps the Tile
            # scheduler (whose own DMA model is far more optimistic) from
            # hoisting a load-gated conversion ahead of urgent PE-coupled
            # work.
            return (2.2 + 1.456 * (2 * (3 * b + c) + half + 1)) / 1000.0

        for b in range(BL):
            bcmu, nhs_col = bcmus[b], nhs_cols[b]
            # ---- filterbanks: Ty|Tx fused per chunk, unnormalized ------
            T = []
            cs_ps = pscs.tile([1, 2 * N], F32)
            for k in range(KC):
                sq_t = sqtmp.tile([128, 2 * N], F32)
                nc.scalar.activation(sq_t, bcmu, ACTF.Square,
                                     bias=nbias[:, k:k + 1])
                T_t = tban.tile([128, 2 * N], F16)
                nc.scalar.activation(T_t, sq_t, ACTF.Exp,
                                     scale=nhs_col[:, 0:1])
                T.append(T_t)
                nc.tensor.matmul(cs_ps, ones_k, T_t,
                                 start=(k == 0), stop=(k == KC - 1))
            # normalizer rows on partition 0: ry = gamma/(csy+eps) via
            # exp(-ln(cs+eps) + p4)  (gamma = exp(p4), so ln gamma = p4),
            # rx = 1/(csx+eps)
            lnrow = rows.tile([1, 2 * N], F32)
            nc.scalar.activation(lnrow, cs_ps, ACTF.Ln, bias=small1[:, :])
            ry = rows.tile([1, N], F32)
            nc.scalar.activation(ry, lnrow[0:1, 0:N], ACTF.Exp, scale=-1.0,
                                 bias=pt0[0:1, b, 4:5])
            rx = rows.tile([1, N], F32)
            nc.scalar.activation(rx, lnrow[0:1, N:2 * N], ACTF.Exp, scale=-1.0)
            # broadcast both rows to [128, 2, N] (each half identical) so
            # they fold multiplicatively into the f_t copy / output scale
            invy2 = invp.tile([128, 2, N], F32)
            invx2 = invp.tile([128, 2, N], F32)
            for j in range(2):
                nc.gpsimd.partition_broadcast(invy2[:, j], ry)
                nc.gpsimd.partition_broadcast(invx2[:, j], rx)

            # fp32->fp16 conversions: first half (hc 0-1) on DVE, second
            # half on the otherwise-idle Pool engine, each stamped with
            # its half-tile's estimated DMA arrival time
            xt_row = []
            for c in range(C):
                xt = xbfp.tile([128, KC, W], F16)
                with tc.tile_wait_until(load_ready_ms(b, c, 0)):
                    nc.vector.tensor_copy(xt[:, 0:2], xf_tiles[b][c][:, 0:2])
                with tc.tile_wait_until(load_ready_ms(b, c, 1)):
                    nc.gpsimd.tensor_copy(xt[:, 2:4], xf_tiles[b][c][:, 2:4])
                xt_row.append(xt)

            # ---- glimpse read: two chained GEMMs per channel -----------
            for c in range(C):
                xt = xt_row[c]
                fyx = []
                for j in range(2):  # wc pairs
                    p1 = ps1.tile([128, 2 * N], F32)
                    for half in range(2):
                        wc = 2 * j + half
                        for hc in range(KC):
                            nc.tensor.matmul(
                                p1[:, half * N:(half + 1) * N],
                                xt[:, hc, wc * 128:(wc + 1) * 128],
                                T[hc][:, 0:N],
                                start=(hc == 0), stop=(hc == KC - 1))
                    # PSUM->SBUF with the y-normalizer (and gamma) folded in
                    f_t = fyxp.tile([128, 2 * N], F16)
                    nc.vector.tensor_tensor(
                        f_t, p1, invy2.rearrange("p a b -> p (a b)"), ALU.mult)
                    fyx.append(f_t)
                ot = outp.tile([128, 2, N], F16)
                p2 = ps2.tile([128, 2, N], F32, tag="p2")
                for nch in range(2):
                    for wc in range(KC):
                        nc.tensor.matmul(
                            p2[:, nch, :],
                            fyx[wc // 2][:, (wc % 2) * N + nch * 128:
                                         (wc % 2) * N + (nch + 1) * 128],
                            T[wc][:, N:2 * N],
                            start=(wc == 0), stop=(wc == KC - 1))
                # single fused output scale: *= invx (both nch halves)
                nc.vector.tensor_tensor(
                    ot.rearrange("p a b -> p (a b)"),
                    p2.rearrange("p a b -> p (a b)"),
                    invx2.rearrange("p a b -> p (a b)"), ALU.mult)
                nc.sync.dma_start(
                    out=o_d[b, c].rearrange("(nch p) m -> p nch m", p=128), in_=ot)


_NC_CACHE = None


def _build():
    global _NC_CACHE
    if _NC_CACHE is None:
        nc = bacc.Bacc("TRN2", target_bir_lowering=False, debug=False,
                       enable_asserts=False, num_devices=NCORES)
        with tile.TileContext(nc) as tc:
            _kernel_body(tc)
        # Steer bacc's greedy ACT table-set choice to the one set that has
        # Exp+Ln+Square+Copy+Identity, else every per-batch Ln costs two
        # ~2.7us table reloads. Only the selection input is patched — set
        # ids and on-device table contents are untouched.
        ours = {ACTF.Exp, ACTF.Ln, ACTF.Square, ACTF.Copy, ACTF.Identity}
        keep = "natural_log_exp_and_others"
        orig = bacc.get_activation_tables

        def steered(arch):
            return {k: (v if k == keep else set(v) - ours)
                    for k, v in orig(arch).items()}

        bacc.get_activation_tables = steered
        try:
            nc.compile()
        finally:
            bacc.get_activation_tables = orig
        _NC_CACHE = nc
    return _NC_CACHE


def _run(x, p, trace=False, **kw):
    nc = _build()
    x = np.ascontiguousarray(x, dtype=np.float32)
    p = np.ascontiguousarray(p, dtype=np.float32)
    assert x.shape == (B, C, H, W) and p.shape == (B, 5), (x.shape, p.shape)
    in_maps = [
        {"x": x[i * BL:(i + 1) * BL], "p": p[i * BL:(i + 1) * BL]}
        for i in range(NCORES)
    ]
    res = run_bass_kernel_spmd(nc, in_maps, list(range(NCORES)), trace=trace, **kw)
    out = np.concatenate(
        [np.asarray(res.results[i]["out"], dtype=np.float32) for i in range(NCORES)],
        axis=0)
    return out, res


def kernel(x, p):
    out, _ = _run(x, p)
    return out
